# revision 1
# baseline (speedup 1.0000x reference)
"""Trainium2 Bass kernel for nn_CausalUpsamplingLRU — v2 (fp16 datapath).

32 autoregressive passes of a diagonal complex LRU over S=512, output fed
back as input. Data-parallel over batch (B=8 -> 8 cores).

v2 changes vs baseline:
 - fp16 everywhere on the elementwise path (DVE 2x mode) + fp16 matmuls,
   with a global 2^-13 prescale on x (system is linear; host unscales).
   Error budget: signal grows ~1.6x/pass so per-pass rounding ~5e-4 ->
   final ~2e-3, well under the 2e-2 gate.
 - Single weight piece (no hi+lo split): halves PE work.
 - Gauss 3-product rotate-out: k1=cos*(s_re+s_im), k2=s_re*(sin-cos),
   k3=s_im*(cos+sin); y = (Cre-Cim)k1 + (-Cim)k2 + (-Cre)k3: 3 C-products
   per n-tile instead of 4.
 - Paired [128,1024] elementwise ops via a 3-half u buffer (u_re|u_im|u_re).
 - Activation engine drains PSUM->fp16 SBUF (u and x copies), freeing DVE.
 - Scans + ksum on Pool (gpsimd), products on DVE; last pass computes only
   the final column of the output projection.
"""
import sys
if '/opt/trn_rl_repo' not in sys.path:
    sys.path.insert(0, '/opt/trn_rl_repo')
import numpy as np
import concourse.bass as bass
import concourse.tile as tile
from concourse import bacc, mybir
from concourse.bass_utils import run_bass_kernel_spmd

F32 = mybir.dt.float32
FP16 = mybir.dt.float16
OP = mybir.AluOpType
ACT_COPY = mybir.ActivationFunctionType.Copy
ACT_IDENT = mybir.ActivationFunctionType.Identity

B_SZ, SEQ, IN_CH, OUT_CH, STATE, OUT_SEQ = 8, 512, 256, 256, 384, 32
NT = STATE // 128   # 3 n-tiles
CT = IN_CH // 128   # 2 c-chunks
OT = OUT_CH // 128  # 2 o-tiles
PRESCALE = 2.0 ** -13

# engine assignment per op slot: 'v' = vector(DVE), 'g' = gpsimd(Pool);
# value is a string of 3 chars (one per n-tile jn)
ENG = dict(P1='vvv', P2a='ggg', P2b='ggv', W='vvv', SR='vvv', SI='vvv',
           KS='ggg', KK='vvv', K1='vvv')
N_LOOP = 1
# PE warmth fillers: zero-adding matmuls ([128, FILL_COLS] each) emitted into
# the y accumulation chain after each n-tile's C products. They execute during
# the wait for the next n-tile's rotated products, keeping the tensor engine
# at full p-state so the critical tail matmuls run warm.
FILLERS = (0, 0, 0)   # after jn=0, jn=1, jn=2's C matmuls
FILL_COLS = 256

_BUILD_CACHE = {}


def _build_nc():
    key = (tuple(sorted(ENG.items())), N_LOOP)
    if key in _BUILD_CACHE:
        return _BUILD_CACHE[key]
    nc = bacc.Bacc("TRN2", target_bir_lowering=False, debug=False)

    # ---- DRAM I/O ----
    xT_d = nc.dram_tensor("xT", [IN_CH, SEQ], FP16, kind="ExternalInput")
    bgre_d = nc.dram_tensor("BgReT", [IN_CH, STATE], FP16, kind="ExternalInput")
    bgim_d = nc.dram_tensor("BgImT", [IN_CH, STATE], FP16, kind="ExternalInput")
    wk1_d = nc.dram_tensor("WK1", [STATE, OUT_CH], FP16, kind="ExternalInput")
    wk2_d = nc.dram_tensor("WK2", [STATE, OUT_CH], FP16, kind="ExternalInput")
    wk3_d = nc.dram_tensor("WK3", [STATE, OUT_CH], FP16, kind="ExternalInput")
    dt_d = nc.dram_tensor("DT", [IN_CH, OUT_CH], FP16, kind="ExternalInput")
    cos_d = nc.dram_tensor("COS", [STATE, SEQ], FP16, kind="ExternalInput")
    cc2_d = nc.dram_tensor("COS2", [STATE, 2 * SEQ], FP16, kind="ExternalInput")
    sn2_d = nc.dram_tensor("SN2", [STATE, 2 * SEQ], FP16, kind="ExternalInput")
    t12_d = nc.dram_tensor("T12", [STATE, 2 * SEQ], FP16, kind="ExternalInput")
    mcol_d = nc.dram_tensor("MCOL", [STATE, 1], F32, kind="ExternalInput")
    c512_d = nc.dram_tensor("C512", [STATE, 1], F32, kind="ExternalInput")
    s512_d = nc.dram_tensor("S512", [STATE, 1], F32, kind="ExternalInput")
    s512n_d = nc.dram_tensor("S512N", [STATE, 1], F32, kind="ExternalInput")
    out_d = nc.dram_tensor("OUT", [OUT_CH, OUT_SEQ], F32, kind="ExternalOutput")

    with tile.TileContext(nc) as tc:
        with tc.tile_pool(name="const", bufs=1) as cp, \
             tc.tile_pool(name="xp", bufs=4) as xp, \
             tc.tile_pool(name="up", bufs=6, space="PSUM") as up, \
             tc.tile_pool(name="yp", bufs=2, space="PSUM") as yp, \
             tc.tile_pool(name="u3p", bufs=4) as u3p, \
             tc.tile_pool(name="pwp", bufs=9) as pwp, \
             tc.tile_pool(name="sp", bufs=4) as sp, \
             tc.tile_pool(name="kp", bufs=9) as kp, \
             tc.tile_pool(name="cyp", bufs=24) as cyp:

            # ---- persistent constants ----
            bgre = [cp.tile([128, STATE], FP16, tag=f"bgre{j}", name=f"bgre{j}")
                    for j in range(CT)]
            bgim = [cp.tile([128, STATE], FP16, tag=f"bgim{j}", name=f"bgim{j}")
                    for j in range(CT)]
            wk1 = [cp.tile([128, OUT_CH], FP16, tag=f"wk1{j}", name=f"wk1{j}")
                   for j in range(NT)]
            wk2 = [cp.tile([128, OUT_CH], FP16, tag=f"wk2{j}", name=f"wk2{j}")
                   for j in range(NT)]
            wk3 = [cp.tile([128, OUT_CH], FP16, tag=f"wk3{j}", name=f"wk3{j}")
                   for j in range(NT)]
            dtw = [cp.tile([128, OUT_CH], FP16, tag=f"dtw{j}", name=f"dtw{j}")
                   for j in range(CT)]
            cost = [cp.tile([128, SEQ], FP16, tag=f"cos{j}", name=f"cos{j}") for j in range(NT)]
            cc2 = [cp.tile([128, 2 * SEQ], FP16, tag=f"cc2{j}", name=f"cc2{j}") for j in range(NT)]
            sn2 = [cp.tile([128, 2 * SEQ], FP16, tag=f"sn2{j}", name=f"sn2{j}") for j in range(NT)]
            t12 = [cp.tile([128, 2 * SEQ], FP16, tag=f"t12{j}", name=f"t12{j}") for j in range(NT)]
            mcol = [cp.tile([128, 1], F32, tag=f"mcol{j}", name=f"mcol{j}") for j in range(NT)]
            c512 = [cp.tile([128, 1], F32, tag=f"c512{j}", name=f"c512{j}") for j in range(NT)]
            s512 = [cp.tile([128, 1], F32, tag=f"s512{j}", name=f"s512{j}") for j in range(NT)]
            s512n = [cp.tile([128, 1], F32, tag=f"s512n{j}", name=f"s512n{j}") for j in range(NT)]
            outb = [cp.tile([128, OUT_SEQ], F32, tag=f"outb{j}", name=f"outb{j}") for j in range(OT)]
            zw = cp.tile([128, 128], FP16, tag="zw", name="zw")
            nc.vector.memset(zw[:], 0.0)

            for j in range(CT):
                nc.sync.dma_start(out=bgre[j][:], in_=bgre_d[j*128:(j+1)*128, :])
                nc.sync.dma_start(out=bgim[j][:], in_=bgim_d[j*128:(j+1)*128, :])
                nc.sync.dma_start(out=dtw[j][:], in_=dt_d[j*128:(j+1)*128, :])
            for j in range(NT):
                nc.sync.dma_start(out=wk1[j][:], in_=wk1_d[j*128:(j+1)*128, :])
                nc.sync.dma_start(out=wk2[j][:], in_=wk2_d[j*128:(j+1)*128, :])
                nc.sync.dma_start(out=wk3[j][:], in_=wk3_d[j*128:(j+1)*128, :])
                nc.sync.dma_start(out=cost[j][:], in_=cos_d[j*128:(j+1)*128, :])
                nc.sync.dma_start(out=cc2[j][:], in_=cc2_d[j*128:(j+1)*128, :])
                nc.sync.dma_start(out=sn2[j][:], in_=sn2_d[j*128:(j+1)*128, :])
                nc.sync.dma_start(out=t12[j][:], in_=t12_d[j*128:(j+1)*128, :])
                nc.sync.dma_start(out=mcol[j][:], in_=mcol_d[j*128:(j+1)*128, :])
                nc.sync.dma_start(out=c512[j][:], in_=c512_d[j*128:(j+1)*128, :])
                nc.sync.dma_start(out=s512[j][:], in_=s512_d[j*128:(j+1)*128, :])
                nc.sync.dma_start(out=s512n[j][:], in_=s512n_d[j*128:(j+1)*128, :])

            def E(slot, jn):
                return nc.vector if ENG[slot][jn] == 'v' else nc.gpsimd

            def tt(slot, jn, out_ap, a_ap, b_ap, op):
                # Pool only supports plain TensorTensor (no TensorScalarPtr)
                eng = nc.vector if ENG[slot][jn] == 'v' else nc.gpsimd
                eng.tensor_tensor(out_ap, a_ap, b_ap, op)

            def emit_body():
                xa = [xp.tile([128, SEQ], FP16, tag="x", name="x") for _ in range(CT)]
                for j in range(CT):
                    nc.sync.dma_start(out=xa[j][:], in_=xT_d[j*128:(j+1)*128, :])

                carry_re = [None]*NT
                carry_im = [None]*NT

                for it in range(OUT_SEQ):
                    last = (it == OUT_SEQ - 1)
                    # columns of the output projection actually needed
                    ysl = slice(SEQ-1, SEQ) if last else slice(0, SEQ)

                    # ---- U matmuls ----
                    u_re, u_im = [], []
                    for jn in range(NT):
                        ur = up.tile([128, SEQ], F32, tag="u", name="u")
                        ui = up.tile([128, SEQ], F32, tag="u", name="u")
                        for jc in range(CT):
                            nc.tensor.matmul(ur[:], bgre[jc][:, jn*128:(jn+1)*128],
                                             xa[jc][:], start=(jc == 0),
                                             stop=(jc == CT-1))
                        for jc in range(CT):
                            nc.tensor.matmul(ui[:], bgim[jc][:, jn*128:(jn+1)*128],
                                             xa[jc][:], start=(jc == 0),
                                             stop=(jc == CT-1))
                        u_re.append(ur); u_im.append(ui)

                    # ---- y PSUM: D x accumulates first ----
                    yps = []
                    for jo in range(OT):
                        y = yp.tile([128, SEQ], F32, tag="y", name="y")
                        for jc in range(CT):
                            nc.tensor.matmul(y[:, ysl], dtw[jc][:, jo*128:(jo+1)*128],
                                             xa[jc][:, ysl], start=(jc == 0),
                                             stop=False)
                        yps.append(y)

                    # ---- software-pipelined by one n-tile: emit
                    # rotate-in(jn) before rotate-out(jn-1), so the in-order
                    # DVE queue never stalls on a scan (the scan of jn-1 runs
                    # on Pool while DVE does rotate-in(jn)), while C matmuls
                    # still fire incrementally per n-tile.
                    svals = []

                    def emit_rotin(jn):
                        u2 = u3p.tile([128, 2*SEQ], FP16, tag="u2", name="u2")
                        nc.scalar.activation(u2[:, 0:SEQ], u_re[jn][:], ACT_COPY)
                        nc.scalar.activation(u2[:, SEQ:2*SEQ], u_im[jn][:], ACT_COPY)
                        # P1 = (cos|cos) * (u_re|u_im) = (t1|t3)
                        # P2 = (sin*u_im | -sin*u_re) = (t2|t4)
                        # W  = P1 + P2 = (w_re|w_im)
                        p1 = pwp.tile([128, 2*SEQ], FP16, tag="pw", name="pw")
                        p2 = pwp.tile([128, 2*SEQ], FP16, tag="pw", name="pw")
                        w = pwp.tile([128, 2*SEQ], FP16, tag="pw", name="pw")
                        tt('P1', jn, p1[:], cc2[jn][:], u2[:], OP.mult)
                        tt('P2a', jn, p2[:, 0:SEQ], sn2[jn][:, 0:SEQ], u2[:, SEQ:2*SEQ], OP.mult)
                        tt('P2b', jn, p2[:, SEQ:2*SEQ], sn2[jn][:, SEQ:2*SEQ], u2[:, 0:SEQ], OP.mult)
                        tt('W', jn, w[:], p1[:], p2[:], OP.add)
                        # real scans: s_t = m s_{t-1} + w_t
                        s = sp.tile([128, 2*SEQ], FP16, tag="s", name="s")
                        d0 = mcol[jn][:].broadcast_to((128, SEQ))
                        init_r = 0.0 if it == 0 else carry_re[jn][:]
                        init_i = 0.0 if it == 0 else carry_im[jn][:]
                        E('SR', jn).tensor_tensor_scan(s[:, 0:SEQ], d0, w[:, 0:SEQ],
                                                       init_r, OP.mult, OP.add)
                        E('SI', jn).tensor_tensor_scan(s[:, SEQ:2*SEQ], d0, w[:, SEQ:2*SEQ],
                                                       init_i, OP.mult, OP.add)
                        svals.append(s)

                    def emit_rotout(jn):
                        s = svals[jn]
                        ysl2 = slice(SEQ + ysl.start, SEQ + ysl.stop)
                        ks = kp.tile([128, SEQ], FP16, tag="k", name="k")
                        k1 = kp.tile([128, SEQ], FP16, tag="k", name="k")
                        kk = kp.tile([128, 2*SEQ], FP16, tag="kk", name="kk")
                        tt('KS', jn, ks[:, ysl], s[:, ysl], s[:, ysl2], OP.add)
                        tt('K1', jn, k1[:, ysl], cost[jn][:, ysl], ks[:, ysl], OP.mult)
                        if last:
                            tt('KK', jn, kk[:, ysl], t12[jn][:, ysl], s[:, ysl], OP.mult)
                            tt('KK', jn, kk[:, ysl2], t12[jn][:, ysl2], s[:, ysl2], OP.mult)
                        else:
                            tt('KK', jn, kk[:], t12[jn][:], s[:], OP.mult)
                        for jo in range(OT):
                            o0 = jo*128
                            fin = (jn == NT-1)
                            nc.tensor.matmul(yps[jo][:, ysl], wk1[jn][:, o0:o0+128],
                                             k1[:, ysl], start=False, stop=False)
                            nc.tensor.matmul(yps[jo][:, ysl], wk2[jn][:, o0:o0+128],
                                             kk[:, ysl], start=False, stop=False)
                            nc.tensor.matmul(yps[jo][:, ysl], wk3[jn][:, o0:o0+128],
                                             kk[:, ysl2], start=False, stop=fin)

                    emit_rotin(0)
                    emit_rotin(1)
                    emit_rotout(0)
                    emit_rotin(2)
                    emit_rotout(1)
                    emit_rotout(2)

                    # ---- carries to next iteration (after all u2 copies in
                    # the ACT queue, so they don't delay them) ----
                    if not last:
                        for jn in range(NT):
                            s = svals[jn]
                            q = cyp.tile([128, 1], F32, tag="cy", name="cy")
                            crn = cyp.tile([128, 1], F32, tag="cy", name="cy")
                            nc.scalar.activation(q[:], s[:, 2*SEQ-1:2*SEQ], ACT_COPY,
                                                 scale=s512n[jn][:])
                            nc.scalar.activation(crn[:], s[:, SEQ-1:SEQ], ACT_IDENT,
                                                 scale=c512[jn][:], bias=q[:])
                            r2 = cyp.tile([128, 1], F32, tag="cy", name="cy")
                            cin = cyp.tile([128, 1], F32, tag="cy", name="cy")
                            nc.scalar.activation(r2[:], s[:, SEQ-1:SEQ], ACT_COPY,
                                                 scale=s512[jn][:])
                            nc.scalar.activation(cin[:], s[:, 2*SEQ-1:2*SEQ], ACT_IDENT,
                                                 scale=c512[jn][:], bias=r2[:])
                            carry_re[jn] = crn; carry_im[jn] = cin

                    # ---- outputs + next x ----
                    xa_next = []
                    for jo in range(OT):
                        if not last:
                            xn = xp.tile([128, SEQ], FP16, tag="x", name="x")
                            nc.scalar.activation(xn[:], yps[jo][:], ACT_COPY)
                            xa_next.append(xn)
                        # last column -> output buffer, off the ACT queue (DVE
                        # is idle at the pass boundary)
                        nc.vector.tensor_scalar_add(outb[jo][:, it:it+1],
                                                    yps[jo][:, SEQ-1:SEQ], 0.0)
                    if not last:
                        xa = xa_next

            if N_LOOP > 1:
                with tc.For_i(0, N_LOOP, 1) as _i:
                    emit_body()
            else:
                emit_body()

            for jo in range(OT):
                nc.sync.dma_start(out=out_d[jo*128:(jo+1)*128, :], in_=outb[jo][:])
    nc.compile()
    _BUILD_CACHE[key] = nc
    return nc


def _host_precompute(x, nu_log, theta_log, gamma_log, B_re, B_im, C_re, C_im, D):
    f8 = np.float64
    nu_log = np.asarray(nu_log, f8); theta_log = np.asarray(theta_log, f8)
    gamma_log = np.asarray(gamma_log, f8)
    B_re = np.asarray(B_re, f8); B_im = np.asarray(B_im, f8)
    C_re = np.asarray(C_re, f8); C_im = np.asarray(C_im, f8)
    D = np.asarray(D, f8)
    m = np.exp(-np.exp(nu_log)); theta = np.exp(theta_log)
    gamma = np.exp(gamma_log)
    t = np.arange(1, SEQ + 1, dtype=f8)[None, :]
    ang = theta[:, None] * t
    cos = np.cos(ang); sin = np.sin(ang)
    f4, f2 = np.float32, np.float16

    common = dict(
        BgReT=(gamma[:, None]*B_re).T.astype(f2),
        BgImT=(gamma[:, None]*B_im).T.astype(f2),
        WK1=(C_re - C_im).T.astype(f2),
        WK2=(-C_im).T.astype(f2),
        WK3=(-C_re).T.astype(f2),
        DT=D.T.astype(f2),
        COS=cos.astype(f2),
        COS2=np.concatenate([cos, cos], axis=1).astype(f2),
        SN2=np.concatenate([sin, -sin], axis=1).astype(f2),
        T12=np.concatenate([sin - cos, cos + sin], axis=1).astype(f2),
        MCOL=m.astype(f4)[:, None],
        C512=np.cos(theta*SEQ).astype(f4)[:, None],
        S512=np.sin(theta*SEQ).astype(f4)[:, None],
        S512N=(-np.sin(theta*SEQ)).astype(f4)[:, None],
    )
    x = np.asarray(x, np.float64) * PRESCALE
    in_maps = []
    for b in range(B_SZ):
        im = dict(common)
        im['xT'] = np.ascontiguousarray(x[b].T).astype(f2)
        in_maps.append(im)
    return in_maps


def kernel(x, nu_log, theta_log, gamma_log, B_re, B_im, C_re, C_im, D):
    nc = _build_nc()
    in_maps = _host_precompute(x, nu_log, theta_log, gamma_log,
                               B_re, B_im, C_re, C_im, D)
    res = run_bass_kernel_spmd(nc, in_maps, list(range(B_SZ)))
    out = np.stack([res.results[b]['OUT'].T for b in range(B_SZ)], axis=0)
    return np.ascontiguousarray((out / PRESCALE).astype(np.float32))

